# revision 1
# baseline (speedup 1.0000x reference)
"""Trainium2 Bass kernel for nn_GroupLinearEncoder.

Math (reference):
  h_b = feat_proj(x_b) = BN(einsum over l,c of x_b and w1_b, w2_b)   (N,1024)
  latent = 0.5*(bn(h0)+bn(h1))
  group_pred = (latent @ shared_w.T) @ embed_w.T + embed_b
  subj       = einsum(latent, fc_w[indices]) + b_sel
  out        = group_pred + subj @ embed_w.T + embed_b
             = (latent @ shared_w.T + subj) @ embed_w.T + 2*embed_b

Key algebraic folds used here:
  * group_pred + subj_res share the embed matmul: z = latent@shared_w.T + subj,
    out = z @ embed_w.T + 2*embed_b  -> embed_w is read ONCE.
  * Because every sample belongs to exactly one group, per-core
    cwt_i = shared_w.T + fc_w.T[:, group_i] applied to mask-selected samples
    and AllReduced over cores yields z directly (shared term included).

Sharding over 8 cores:
  * feat_proj: data-parallel over batch (8 samples/core).  Branch 1 (smaller)
    runs first so its h AllGather hides under branch 0's compute; branch 0's
    AllGather is the only exposed collective before BN.
  * z: group-parallel (core i handles group i via sample masks), computed
    transposed (z.T, 512-wide matmuls), AllReduce, then PE-transpose back.
  * embed: column-parallel over out_dim (4944 rows/core, padded), concat on
    host.  embed_w is fully resident in SBUF via three staged pools so the
    final matmul is tensor-bound, with prefetch running from t=0.
"""

import os
import sys

import numpy as np

N, H, P, KE = 64, 1024, 2048, 39548
PT = P // 128
NCORES = 8
NS = N // NCORES            # samples per core
L0, C0 = 257, 1024
L1, C1 = 197, 768
W = 4944                    # embed rows per core (8*4944 = 39552, 4 pad)
BN_EPS = 1e-5

_CACHE = {}


def _build_nc():
    if "/opt/trn_rl_repo" not in sys.path:
        sys.path.insert(0, "/opt/trn_rl_repo")
    import concourse.bass as bass
    import concourse.tile as tile
    from concourse import bacc, mybir
    from contextlib import ExitStack

    f32 = mybir.dt.float32
    bf16 = mybir.dt.bfloat16
    ALU = mybir.AluOpType
    ACTF = mybir.ActivationFunctionType

    nc = bacc.Bacc(num_devices=NCORES)

    KT = H // 128            # 8 k-tiles
    NB0 = C0 // 128          # 8 c-chunks branch0
    NB1 = C1 // 128          # 6 c-chunks branch1
    NBLK = (W + 511) // 512  # embed col blocks

    x0t = nc.declare_dram_parameter("x0t", [C0, NS, L0], bf16, isOutput=False)
    x1t = nc.declare_dram_parameter("x1t", [C1, NS, L1], bf16, isOutput=False)
    w2_0t = nc.declare_dram_parameter("w2_0t", [C0, H], bf16, isOutput=False)
    w2_1t = nc.declare_dram_parameter("w2_1t", [C1, H], bf16, isOutput=False)
    w1_0 = nc.declare_dram_parameter("w1_0", [H, L0], f32, isOutput=False)
    w1_1 = nc.declare_dram_parameter("w1_1", [H, L1], f32, isOutput=False)
    gam0 = nc.declare_dram_parameter("gam0", [128, 8], f32, isOutput=False)
    bet0 = nc.declare_dram_parameter("bet0", [128, 8], f32, isOutput=False)
    gam1 = nc.declare_dram_parameter("gam1", [128, 8], f32, isOutput=False)
    bet1 = nc.declare_dram_parameter("bet1", [128, 8], f32, isOutput=False)
    cwt = nc.declare_dram_parameter("cwt", [H, P], bf16, isOutput=False)
    fcb = nc.declare_dram_parameter("fcb", [1, P], bf16, isOutput=False)
    maskrow = nc.declare_dram_parameter("maskrow", [1, N], bf16, isOutput=False)
    mask = nc.declare_dram_parameter("mask", [128, N], f32, isOutput=False)
    ewt = nc.declare_dram_parameter("ewt", [P, W], bf16, isOutput=False)
    eb2 = nc.declare_dram_parameter("eb2", [1, W], bf16, isOutput=False)
    out = nc.declare_dram_parameter("out", [N, W], f32, isOutput=True)

    with tile.TileContext(nc) as tc, ExitStack() as stack:
        singles = stack.enter_context(tc.tile_pool(name="singles", bufs=1))
        dpool = stack.enter_context(tc.tile_pool(name="dram", bufs=1, space="DRAM"))
        tpool = stack.enter_context(tc.tile_pool(name="touchp", bufs=2))
        stx = stack.enter_context(tc.tile_pool(name="stats", bufs=1))
        scr = stack.enter_context(tc.tile_pool(name="scr", bufs=2))
        ewp1 = stack.enter_context(tc.tile_pool(name="ewp1", bufs=1))
        _tn = [0]

        def touch(ap):
            # absorb a DMA's queue semaphores into DVE's vector clock so
            # downstream DVE ops need only engine-local ordering
            _tn[0] += 1
            tt = tpool.tile([ap.shape[0], 1], ap.dtype, tag="touch",
                            name=f"touch{_tn[0]}")
            nc.vector.tensor_copy(out=tt, in_=ap[:, 0:1])

        # resident embed weight tiles; pc 0-9 load from t=0 (interleaved with
        # stage A), 10-12 after the branch pools free, 13-15 after cwt frees
        ewsb = [None] * PT

        _gt = [0]

        def emit_ewt(pool, pc, gate=None):
            # embed-weight prefetch rides the scalar-engine HWDGE ring so it
            # never blocks the sync ring's latency-critical stage-A loads.
            # `gate`: a tile whose arrival throttles this DMA's dispatch (via
            # a tiny scalar-engine copy ahead of it in ACT program order) so
            # the prefetch doesn't hog HBM while stage A ramps up.
            if gate is not None:
                _gt[0] += 1
                gg = tpool.tile([1, 1], gate.dtype, tag="gate",
                                name=f"gate{_gt[0]}")
                nc.scalar.copy(out=gg, in_=gate[0:1, 0:1])
            t = pool.tile([128, W], bf16, tag=f"ew{pc}", name=f"ew{pc}")
            nc.scalar.dma_start(out=t, in_=ewt[pc * 128:(pc + 1) * 128, :])
            ewsb[pc] = t

        # --- small resident tensors ---
        h0sb = singles.tile([128, N], f32)       # col = kt*8 + n_local
        h1sb = singles.tile([128, N], f32)
        gam0sb = singles.tile([128, 8], f32)
        bet0sb = singles.tile([128, 8], f32)
        gam1sb = singles.tile([128, 8], f32)
        bet1sb = singles.tile([128, 8], f32)
        masksb = singles.tile([128, N], f32)
        epssb = singles.tile([128, 1], f32)
        hall0 = singles.tile([128, NCORES, N], f32)
        hall1 = singles.tile([128, NCORES, N], f32)
        lmsb = []
        for kt in range(KT):
            lmsb.append(singles.tile([128, N], bf16, tag=f"lm{kt}",
                                     name=f"lm{kt}"))

        # dram buffers for collectives
        hb1 = dpool.tile([128, N], f32)
        hg1 = dpool.tile([NCORES, 128, N], f32, addr_space="Shared")
        hb0 = dpool.tile([128, N], f32)
        hg0 = dpool.tile([NCORES, 128, N], f32, addr_space="Shared")
        zfc = dpool.tile([PT, 128, N], bf16)
        zr = dpool.tile([PT, 128, N], bf16, addr_space="Shared")

        def feat_branch(xt, w2t, w1p, NB, L, hsb, tagp, two_per_mm,
                        ew_slots, ew_pool):
            """One feat_proj branch: h[k, n] = sum_l w1[k,l] sum_c w2[k,c] x[n,l,c].
            Returns after filling hsb.  ew_slots: {kt: [pc...]} embed-weight
            DMAs to interleave at that kt boundary."""
            def w2load(kt):
                blk = []
                for ci in range(NB):
                    t = brp.tile([128, 128], bf16, tag=f"{tagp}w2_{ci}", bufs=2,
                                 name=f"{tagp}w2_{kt}_{ci}")
                    nc.sync.dma_start(
                        out=t, in_=w2t[ci * 128:(ci + 1) * 128,
                                       kt * 128:(kt + 1) * 128])
                    blk.append(t)
                return blk

            # first weight block + x chunks ahead of w1 so the first matmuls
            # can issue as early as possible
            pend = [w2load(0)]
            xsb = []
            for ci in range(NB):
                t = brp.tile([128, NS, L], bf16, tag=f"{tagp}x{ci}",
                             name=f"{tagp}x{ci}")
                nc.sync.dma_start(out=t, in_=xt[ci * 128:(ci + 1) * 128, :, :])
                xsb.append(t)
            pend.append(w2load(1))
            w1sb = []
            for kt in range(KT):
                t = brp.tile([128, L], f32, tag=f"{tagp}w1_{kt}",
                             name=f"{tagp}w1_{kt}")
                nc.sync.dma_start(out=t, in_=w1p[kt * 128:(kt + 1) * 128, :])
                touch(t)
                w1sb.append(t)
            for kt in range(KT):
                w2blk = pend.pop(0)
                if kt + 2 < KT:
                    pend.append(w2load(kt + 2))
                for pc in ew_slots.get(kt, ()):
                    emit_ewt(ew_pool, pc, gate=w2blk[0])
                # all 8 samples accumulate concurrently (8 PSUM banks) so one
                # LDWEIGHTS serves 8 (or 4) matmuls — LDW opt is disabled in
                # this toolchain, so amortization is on us
                if two_per_mm:
                    vs = []
                    for j in range(4):
                        v = pspool.tile([128, 2, L], f32, tag=f"v{j}",
                                        name=f"{tagp}v{kt}_{j}")
                        vs.append(v)
                    for ci in range(NB):
                        for j in range(4):
                            nc.tensor.matmul(
                                out=vs[j][:, :, :],
                                lhsT=w2blk[ci][:, :],
                                rhs=xsb[ci][:, 2 * j:2 * j + 2, :],
                                start=(ci == 0),
                                stop=(ci == NB - 1),
                            )
                    for j in range(4):
                        for s in range(2):
                            n = 2 * j + s
                            col = kt * 8 + n
                            ts = scr.tile([128, L], f32, tag="ascr",
                                          name=f"{tagp}as{kt}_{j}_{s}")
                            nc.vector.scalar_tensor_tensor(
                                out=ts[:, :], in0=vs[j][:, s, :],
                                scalar=1.0, in1=w1sb[kt][:, :],
                                op0=ALU.mult, op1=ALU.mult,
                                accum_out=hsb[:, col:col + 1])
                else:
                    vs = []
                    for j in range(8):
                        v = pspool.tile([128, L], f32, tag=f"v{j}",
                                        name=f"{tagp}v{kt}_{j}")
                        vs.append(v)
                    for ci in range(NB):
                        for j in range(8):
                            nc.tensor.matmul(
                                out=vs[j][:, :],
                                lhsT=w2blk[ci][:, :],
                                rhs=xsb[ci][:, j, :],
                                start=(ci == 0),
                                stop=(ci == NB - 1),
                            )
                    for j in range(8):
                        col = kt * 8 + j
                        ts = scr.tile([128, L], f32, tag="ascr",
                                      name=f"{tagp}as{kt}_{j}")
                        nc.vector.scalar_tensor_tensor(
                            out=ts[:, :], in0=vs[j][:, :],
                            scalar=1.0, in1=w1sb[kt][:, :],
                            op0=ALU.mult, op1=ALU.mult,
                            accum_out=hsb[:, col:col + 1])

        def branch_stats(hall, gsb, bsb, pref):
            """Batched biased batch-norm stats over all 64 samples.
            Returns (a, nb) each [128, 8] (col = kt): a = 0.5*gamma*rstd,
            nb = 0.5*beta - a*mu."""
            s1 = stx.tile([128, KT], f32, name=f"{pref}s1")
            sq = stx.tile([128, KT], f32, name=f"{pref}sq")
            for kt in range(KT):
                hs = hall[:, :, kt * 8:kt * 8 + 8]
                t0 = scr.tile([128, N], f32, tag="ttrscr", name=f"{pref}t0_{kt}")
                t03 = t0[:].rearrange("p (g n) -> p g n", g=NCORES)
                nc.vector.scalar_tensor_tensor(
                    out=t03, in0=hs, scalar=0.0, in1=hs,
                    op0=ALU.add, op1=ALU.bypass,
                    accum_out=s1[:, kt:kt + 1])
                t1 = scr.tile([128, N], f32, tag="ttrscr", name=f"{pref}t1_{kt}")
                t13 = t1[:].rearrange("p (g n) -> p g n", g=NCORES)
                nc.vector.scalar_tensor_tensor(
                    out=t13, in0=hs, scalar=1.0, in1=hs,
                    op0=ALU.mult, op1=ALU.mult,
                    accum_out=sq[:, kt:kt + 1])
            mu = stx.tile([128, KT], f32, name=f"{pref}mu")
            nc.vector.tensor_scalar_mul(mu, s1, 1.0 / N)
            msq = stx.tile([128, KT], f32, name=f"{pref}msq")
            nc.vector.tensor_mul(msq, mu, mu)
            var = stx.tile([128, KT], f32, name=f"{pref}var")
            nc.vector.scalar_tensor_tensor(
                out=var, in0=sq, scalar=1.0 / N, in1=msq,
                op0=ALU.mult, op1=ALU.subtract)
            sd = stx.tile([128, KT], f32, name=f"{pref}sd")
            nc.scalar.activation(out=sd, in_=var, func=ACTF.Sqrt,
                                 bias=epssb, scale=1.0)
            rstd = stx.tile([128, KT], f32, name=f"{pref}rstd")
            nc.vector.reciprocal(out=rstd, in_=sd)
            a = stx.tile([128, KT], f32, name=f"{pref}a")
            nc.vector.tensor_mul(a, rstd, gsb)
            amu = stx.tile([128, KT], f32, name=f"{pref}amu")
            nc.vector.tensor_mul(amu, a, mu)
            nb = stx.tile([128, KT], f32, name=f"{pref}nb")
            nc.vector.tensor_sub(nb, bsb, amu)
            return a, nb

        ps_ctx = tc.tile_pool(name="ps", bufs=1, space="PSUM")
        pspool = ps_ctx.__enter__()

        with tc.tile_pool(name="br1", bufs=1) as br1, \
             tc.tile_pool(name="br0", bufs=1) as br0:
            # ---- stage A: branch 1 first (smaller), then branch 0 ----
            brp = br1
            feat_branch(x1t, w2_1t, w1_1, NB1, L1, h1sb, "b1", True,
                        {}, ewp1)
            # small resident loads (emitted after branch1's so they don't
            # delay its first matmuls)
            nc.sync.dma_start(out=gam0sb, in_=gam0[:, :])
            nc.sync.dma_start(out=bet0sb, in_=bet0[:, :])
            nc.sync.dma_start(out=gam1sb, in_=gam1[:, :])
            nc.sync.dma_start(out=bet1sb, in_=bet1[:, :])
            nc.sync.dma_start(out=masksb, in_=mask[:, :])
            for _t in (gam0sb, bet0sb, gam1sb, bet1sb, masksb):
                touch(_t)
            nc.vector.memset(epssb, BN_EPS)

            brp = br0
            feat_branch(x0t, w2_0t, w1_0, NB0, L0, h0sb, "b0", False,
                        {0: [0, 1], 1: [2], 2: [3], 3: [4], 4: [5], 5: [6],
                         6: [7], 7: [8, 9]}, ewp1)

            # gather h1 while branch 0 computes.  These dependency-gated
            # transfers ride the SWDGE path so they never queue behind the
            # bulk HWDGE loads (x/w2/embed prefetch).
            nc.gpsimd.dma_start(out=hb1, in_=h1sb[:, :])
            nc.gpsimd.collective_compute(
                "AllGather", ALU.bypass,
                replica_groups=[list(range(NCORES))],
                ins=[hb1[:].opt()], outs=[hg1[:].opt()])
            nc.gpsimd.dma_start(
                out=hall1[:, :, :], in_=hg1[:].rearrange("g p n -> p g n"))
            touch(hall1[:, 0, :])

            # branch-1 stats hide under branch 0's compute
            a1, nb1 = branch_stats(hall1, gam1sb, bet1sb, "s1_")

            # hb0 + the AllGather it feeds are on the critical path: use the
            # sync ring (drained by now) for the lowest store latency
            nc.sync.dma_start(out=hb0, in_=h0sb[:, :])
            nc.gpsimd.collective_compute(
                "AllGather", ALU.bypass,
                replica_groups=[list(range(NCORES))],
                ins=[hb0[:].opt()], outs=[hg0[:].opt()])

        ps_ctx.__exit__(None, None, None)

        with tc.tile_pool(name="ewp2", bufs=1) as ewp2, \
             tc.tile_pool(name="dstage", bufs=1) as dst:
            for pc in (10, 11, 12):
                emit_ewt(ewp2, pc)
            # D-stage resident tensors (space freed by branch pools)
            eb2sb = dst.tile([1, W], bf16)
            nc.sync.dma_start(out=eb2sb, in_=eb2[:, :])
            ones1 = dst.tile([1, N], bf16)
            nc.vector.memset(ones1, 1.0)
            zAll = dst.tile([128, PT, N], bf16)

            with tc.tile_pool(name="cpool", bufs=1) as cpool:
                cwsb = []
                for kt in range(KT):
                    t = cpool.tile([128, P], bf16, tag=f"cw{kt}", name=f"cw{kt}")
                    nc.sync.dma_start(out=t, in_=cwt[kt * 128:(kt + 1) * 128, :])
                    cwsb.append(t)
                fcbsb = cpool.tile([1, P], bf16)
                nc.sync.dma_start(out=fcbsb, in_=fcb[:, :])
                maskrsb = cpool.tile([1, N], bf16)
                nc.sync.dma_start(out=maskrsb, in_=maskrow[:, :])
                nc.sync.dma_start(
                    out=hall0[:, :, :], in_=hg0[:].rearrange("g p n -> p g n"))
                touch(hall0[:, 0, :])

                # PE warm-up chain gated on hall0's arrival: spans the BN DVE
                # window so stage C's matmuls start at full clock
                warmsrc = cpool.tile([128, 256], bf16, name="warmsrc")
                nc.vector.memset(warmsrc, 0.0)
                nc.vector.tensor_copy(out=warmsrc[:, 0:1], in_=hall0[:, 0, 0:1])

                # ---- BN -> latent*mask per kt ----
                a0, nb0 = branch_stats(hall0, gam0sb, bet0sb, "s0_")
                cc = stx.tile([128, KT], f32, name="ccomb")
                nc.vector.tensor_add(cc, nb0, nb1)

                # ---- BN latent ops, then stage C in two kt-major passes ----
                # (one PSUM accumulation chain per bank: a start=True zeroes
                # the tile's whole zero region, so chains must not share one)
                with tc.tile_pool(name="zps", bufs=1, space="PSUM") as zps, \
                     tc.tile_pool(name="zsp", bufs=2) as zsp:
                    # shares tag zc0 so the 8 zc chains + warm tile fit 8 banks
                    wt = zps.tile([128, 256], f32, tag="zc0", name="warmps")
                    for i in range(30):
                        nc.tensor.matmul(out=wt[:, :], lhsT=warmsrc[:, 0:128],
                                         rhs=warmsrc[:, :], start=True,
                                         stop=True)
                    for kt in range(KT):
                        t1 = scr.tile([128, N], f32, tag="latscr",
                                      name=f"lat1_{kt}")
                        nc.vector.tensor_scalar(
                            out=t1[:].rearrange("p (g n) -> p g n", g=NCORES),
                            in0=hall1[:, :, kt * 8:kt * 8 + 8],
                            scalar1=a1[:, kt:kt + 1], scalar2=cc[:, kt:kt + 1],
                            op0=ALU.mult, op1=ALU.add)
                        t2 = scr.tile([128, N], f32, tag="latscr2",
                                      name=f"lat2_{kt}")
                        nc.vector.scalar_tensor_tensor(
                            out=t2[:].rearrange("p (g n) -> p g n", g=NCORES),
                            in0=hall0[:, :, kt * 8:kt * 8 + 8],
                            scalar=a0[:, kt:kt + 1],
                            in1=t1[:].rearrange("p (g n) -> p g n", g=NCORES),
                            op0=ALU.mult, op1=ALU.add)
                        nc.vector.tensor_mul(lmsb[kt], t2, masksb)
                    zsbufs = []
                    for half in range(2):
                        zsbuf = zsp.tile([128, 8, N], bf16, tag="zst",
                                         name=f"zst{half}")
                        zsbufs.append(zsbuf)
                        zcs = []
                        for j in range(8):
                            pt = half * 8 + j
                            t = zps.tile([128, N], f32, tag=f"zc{j}",
                                         name=f"zc{pt}")
                            zcs.append(t)
                        for kt in range(KT):
                            for j in range(8):
                                pt = half * 8 + j
                                nc.tensor.matmul(
                                    out=zcs[j][:, :],
                                    lhsT=cwsb[kt][:, pt * 128:(pt + 1) * 128],
                                    rhs=lmsb[kt][:, :],
                                    start=(kt == 0), stop=False,
                                )
                        for j in range(8):
                            pt = half * 8 + j
                            nc.tensor.matmul(
                                out=zcs[j][:, :],
                                lhsT=fcbsb[:, pt * 128:(pt + 1) * 128],
                                rhs=maskrsb[:, :], start=False, stop=True)
                            nc.vector.tensor_copy(out=zsbuf[:, j, :],
                                                  in_=zcs[j][:, :])
                        dstv = zfc[half * 8:(half + 1) * 8].rearrange(
                            "t p c -> p t c")
                        nc.sync.dma_start(out=dstv, in_=zsbuf[:, :, :])

            with tc.tile_pool(name="ewp3", bufs=1) as ewp3:
                for pc in (13, 14, 15):
                    emit_ewt(ewp3, pc)
                nc.gpsimd.collective_compute(
                    "AllReduce", ALU.add,
                    replica_groups=[list(range(NCORES))],
                    ins=[zfc[:].opt()], outs=[zr[:].opt()])
                nc.sync.dma_start(
                    out=zAll[:, :, :], in_=zr[:].rearrange("t p n -> p t n"))
                touch(zAll[:, 0, :])

                # ---- stage D : out = z.T @ ewt + 2*eb ----
                # block pairs run as two concurrent accumulation chains on
                # disjoint PE column groups (tile_position), nearly doubling
                # matmul throughput for the 64-wide stationary operand
                with tc.tile_pool(name="odp", bufs=2, space="PSUM") as odp, \
                     tc.tile_pool(name="osp", bufs=4) as osp:
                    for nb0_ in range(0, NBLK, 2):
                        bA, bB = nb0_, nb0_ + 1
                        bsA, bwA = bA * 512, min(512, W - bA * 512)
                        bsB, bwB = bB * 512, min(512, W - bB * 512)
                        odA = odp.tile([N, 512], f32, tag="odA",
                                       name=f"odA{bA}")
                        odB = odp.tile([128, 512], f32, tag="odB",
                                       name=f"odB{bB}")
                        for pc in range(PT):
                            nc.tensor.matmul(
                                out=odA[:, :bwA],
                                lhsT=zAll[:, pc, :],
                                rhs=ewsb[pc][:, bsA:bsA + bwA],
                                start=(pc == 0), stop=False,
                                tile_position=(0, 0))
                            nc.tensor.matmul(
                                out=odB[64:128, :bwB],
                                lhsT=zAll[:, pc, :],
                                rhs=ewsb[pc][:, bsB:bsB + bwB],
                                start=(pc == 0), stop=False,
                                tile_position=(0, 64))
                        nc.tensor.matmul(
                            out=odA[:, :bwA],
                            lhsT=ones1[:, :],
                            rhs=eb2sb[:, bsA:bsA + bwA],
                            start=False, stop=True, tile_position=(0, 0))
                        nc.tensor.matmul(
                            out=odB[64:128, :bwB],
                            lhsT=ones1[:, :],
                            rhs=eb2sb[:, bsB:bsB + bwB],
                            start=False, stop=True, tile_position=(0, 64))
                        osbA = osp.tile([N, 512], f32, tag="osbA",
                                        name=f"osbA{bA}")
                        nc.vector.tensor_copy(out=osbA[:, :bwA],
                                              in_=odA[:, :bwA])
                        nc.sync.dma_start(out=out[:, bsA:bsA + bwA],
                                          in_=osbA[:, :bwA])
                        osbB = osp.tile([128, 512], f32, tag="osbB",
                                        name=f"osbB{bB}")
                        nc.vector.tensor_copy(out=osbB[64:128, :bwB],
                                              in_=odB[64:128, :bwB])
                        nc.sync.dma_start(out=out[:, bsB:bsB + bwB],
                                          in_=osbB[64:128, :bwB])

    nc.compile()
    return nc


def _host_prep(x0, x1, w1_0, w2_0, gamma0, beta0, w1_1, w2_1, gamma1, beta1,
               shared_w, fc_w, fc_b, embed_w, embed_b, indices):
    import ml_dtypes
    f = np.float32
    bf = ml_dtypes.bfloat16
    x0t = np.ascontiguousarray(x0.transpose(2, 0, 1)).astype(bf)   # [1024, 64, 257]
    x1t = np.ascontiguousarray(x1.transpose(2, 0, 1)).astype(bf)   # [768, 64, 197]
    w2_0t = np.ascontiguousarray(w2_0.T).astype(bf)
    w2_1t = np.ascontiguousarray(w2_1.T).astype(bf)
    gam0 = np.ascontiguousarray((gamma0 * 0.5).reshape(8, 128).T, dtype=f)
    bet0 = np.ascontiguousarray((beta0 * 0.5).reshape(8, 128).T, dtype=f)
    gam1 = np.ascontiguousarray((gamma1 * 0.5).reshape(8, 128).T, dtype=f)
    bet1 = np.ascontiguousarray((beta1 * 0.5).reshape(8, 128).T, dtype=f)
    swt = shared_w.T.astype(f)                                    # [1024, 2048]
    fcwt = fc_w.T.astype(f)                                       # [1024, 16384]
    ewt_pad = np.zeros((P, NCORES * W), dtype=bf)
    ewt_pad[:, :KE] = embed_w.T.astype(bf)
    eb2_pad = np.zeros((1, NCORES * W), dtype=bf)
    eb2_pad[0, :KE] = (2.0 * embed_b).astype(bf)

    idx = np.asarray(indices).astype(np.int64)
    in_maps = []
    for i in range(NCORES):
        m = (idx == i).astype(f)
        in_maps.append({
            "x0t": np.ascontiguousarray(x0t[:, i * NS:(i + 1) * NS, :]),
            "x1t": np.ascontiguousarray(x1t[:, i * NS:(i + 1) * NS, :]),
            "w2_0t": w2_0t,
            "w2_1t": w2_1t,
            "w1_0": np.ascontiguousarray(w1_0, dtype=f),
            "w1_1": np.ascontiguousarray(w1_1, dtype=f),
            "gam0": gam0, "bet0": bet0, "gam1": gam1, "bet1": bet1,
            "cwt": np.ascontiguousarray(swt + fcwt[:, i * P:(i + 1) * P]).astype(bf),
            "fcb": np.ascontiguousarray(fc_b[i * P:(i + 1) * P].reshape(1, P)).astype(bf),
            "maskrow": np.ascontiguousarray(m.reshape(1, N)).astype(bf),
            "mask": np.ascontiguousarray(np.broadcast_to(m, (128, N))),
            "ewt": np.ascontiguousarray(ewt_pad[:, i * W:(i + 1) * W]),
            "eb2": np.ascontiguousarray(eb2_pad[:, i * W:(i + 1) * W]),
        })
    return in_maps


def kernel(**inputs):
    if "/opt/trn_rl_repo" not in sys.path:
        sys.path.insert(0, "/opt/trn_rl_repo")
    from concourse.bass_utils import run_bass_kernel_spmd

    in_maps = _host_prep(**inputs)
    if "nc" not in _CACHE:
        _CACHE["nc"] = _build_nc()
    nc = _CACHE["nc"]
    res = run_bass_kernel_spmd(nc, in_maps, core_ids=list(range(NCORES)))
    outs = [np.asarray(res.results[i]["out"]) for i in range(NCORES)]
    full = np.concatenate(outs, axis=1)[:, :KE]
    return np.ascontiguousarray(full, dtype=np.float32)


if __name__ == "__main__":
    sys.path.insert(0, os.path.dirname(os.path.abspath(__file__)))
    import reference
    inputs = {k: np.asarray(v) for k, v in reference.setup_inputs().items()}
    expected = np.asarray(reference.reference(**inputs))
    actual = kernel(**inputs)
    err = np.abs(actual - expected).max() / (np.abs(expected).max() + 1e-12)
    print("Relative error:", err)



# revision 5
# speedup vs baseline: 1.1628x; 1.1628x over previous
"""Trainium2 Bass kernel for nn_GroupLinearEncoder.

Math (reference):
  h_b = feat_proj(x_b) = BN(einsum over l,c of x_b and w1_b, w2_b)   (N,1024)
  latent = 0.5*(bn(h0)+bn(h1))
  group_pred = (latent @ shared_w.T) @ embed_w.T + embed_b
  subj       = einsum(latent, fc_w[indices]) + b_sel
  out        = group_pred + subj @ embed_w.T + embed_b
             = (latent @ shared_w.T + subj) @ embed_w.T + 2*embed_b

Key algebraic folds used here:
  * group_pred + subj_res share the embed matmul: z = latent@shared_w.T + subj,
    out = z @ embed_w.T + 2*embed_b  -> embed_w is read ONCE.
  * Because every sample belongs to exactly one group, per-core
    cwt_i = shared_w.T + fc_w.T[:, group_i] applied to mask-selected samples
    and AllReduced over cores yields z directly (shared term included).
    The AllReduce is numerically exact: every z element is nonzero on
    exactly one core (mask-disjoint), so the CCE sums x+0.

Sharding over 8 cores:
  * feat_proj: data-parallel over batch (8 samples/core).  Branch 0 (bigger)
    runs FIRST so its h AllGather hides under branch 1's compute; branch 1's
    AllGather is the only exposed h collective.
  * z: group-parallel (core i handles group i via sample masks), computed
    transposed (z.T), AllReduce in [128,16,64] partition-major layout
    (2KB DMA lines), then used directly as stage-D lhsT slices.
  * embed: column-parallel over out_dim (4944 rows/core, padded), concat on
    host.  embed_w is shipped as e3m4 fp8 (x128 scale folded into cwt/fcb as
    1/128) halving its HBM traffic; all 16 k-tiles are SBUF-resident.

DMA layout rules applied: every bulk HBM load uses >=1KB per-partition
lines (w2 loaded as [128,1024] row tiles, not [128,128] blocks).
"""

import os
import sys

import numpy as np

N, H, P, KE = 64, 1024, 2048, 39548
PT = P // 128
NCORES = 8
NS = N // NCORES            # samples per core
L0, C0 = 257, 1024
L1, C1 = 197, 768
W = 4944                    # embed rows per core (8*4944 = 39552, 4 pad)
BN_EPS = 1e-5
EW_SCALE = 128.0            # embed_w is quantized e3m4 at x128; cwt,fcb carry 1/128

_CACHE = {}


def _build_nc():
    if "/opt/trn_rl_repo" not in sys.path:
        sys.path.insert(0, "/opt/trn_rl_repo")
    import concourse.bass as bass
    import concourse.tile as tile
    from concourse import bacc, mybir
    from contextlib import ExitStack

    f32 = mybir.dt.float32
    bf16 = mybir.dt.bfloat16
    f8e3 = mybir.dt.float8e3
    ALU = mybir.AluOpType
    ACTF = mybir.ActivationFunctionType

    nc = bacc.Bacc(num_devices=NCORES)

    KT = H // 128            # 8 k-tiles
    NB0 = C0 // 128          # 8 c-chunks branch0
    NB1 = C1 // 128          # 6 c-chunks branch1
    NBLK = (W + 511) // 512  # embed col blocks

    x0t = nc.declare_dram_parameter("x0t", [C0, NS, L0], bf16, isOutput=False)
    x1t = nc.declare_dram_parameter("x1t", [C1, NS, L1], bf16, isOutput=False)
    w2_0t = nc.declare_dram_parameter("w2_0t", [C0, H], bf16, isOutput=False)
    w2_1t = nc.declare_dram_parameter("w2_1t", [C1, H], bf16, isOutput=False)
    w1_0 = nc.declare_dram_parameter("w1_0", [H, L0], f32, isOutput=False)
    w1_1 = nc.declare_dram_parameter("w1_1", [H, L1], f32, isOutput=False)
    gam0 = nc.declare_dram_parameter("gam0", [128, 8], f32, isOutput=False)
    bet0 = nc.declare_dram_parameter("bet0", [128, 8], f32, isOutput=False)
    gam1 = nc.declare_dram_parameter("gam1", [128, 8], f32, isOutput=False)
    bet1 = nc.declare_dram_parameter("bet1", [128, 8], f32, isOutput=False)
    cwt = nc.declare_dram_parameter("cwt", [H, P], bf16, isOutput=False)
    fcb = nc.declare_dram_parameter("fcb", [1, P], bf16, isOutput=False)
    maskrow = nc.declare_dram_parameter("maskrow", [1, N], bf16, isOutput=False)
    mask = nc.declare_dram_parameter("mask", [128, N], f32, isOutput=False)
    ewt = nc.declare_dram_parameter("ewt", [P, W], f8e3, isOutput=False)
    eb2 = nc.declare_dram_parameter("eb2", [1, W], bf16, isOutput=False)
    out = nc.declare_dram_parameter("out", [N, W], f32, isOutput=True)

    with tile.TileContext(nc) as tc, ExitStack() as stack:
        singles = stack.enter_context(tc.tile_pool(name="singles", bufs=1))
        dpool = stack.enter_context(tc.tile_pool(name="dram", bufs=1, space="DRAM"))
        tpool = stack.enter_context(tc.tile_pool(name="touchp", bufs=2))
        stx = stack.enter_context(tc.tile_pool(name="stats", bufs=1))
        scr = stack.enter_context(tc.tile_pool(name="scr", bufs=2))
        ewp = stack.enter_context(tc.tile_pool(name="ewp", bufs=1))
        _tn = [0]

        def touch(ap):
            # absorb a DMA's queue semaphores into DVE's vector clock so
            # downstream DVE ops need only engine-local ordering
            _tn[0] += 1
            tt = tpool.tile([ap.shape[0], 1], ap.dtype, tag="touch",
                            name=f"touch{_tn[0]}")
            nc.vector.tensor_copy(out=tt, in_=ap[:, 0:1])

        # resident embed weight tiles (fp8: all 16 fit) + cwt row tiles;
        # both ride the scalar-engine HWDGE ring, gated on stage-A compute
        # progress so they never starve the sync ring's x/w2 feeds.
        ewsb = [None] * PT
        _gt = [0]

        def gated_dma(pool, dst_shape, dtype, src_ap, tag, gate=None):
            if gate is not None:
                _gt[0] += 1
                gg = tpool.tile([1, 1], gate.dtype, tag="gate",
                                name=f"gate{_gt[0]}")
                nc.scalar.copy(out=gg, in_=gate[0:1, 0:1])
            t = pool.tile(dst_shape, dtype, tag=tag, name=tag)
            nc.scalar.dma_start(out=t, in_=src_ap)
            return t

        # --- small resident tensors ---
        h0sb = singles.tile([128, N], f32)       # col = kt*8 + n_local
        h1sb = singles.tile([128, N], f32)
        gam0sb = singles.tile([128, 8], f32)
        bet0sb = singles.tile([128, 8], f32)
        gam1sb = singles.tile([128, 8], f32)
        bet1sb = singles.tile([128, 8], f32)
        masksb = singles.tile([128, N], f32)
        epssb = singles.tile([128, 1], f32)
        hall0 = singles.tile([128, NCORES, N], f32)
        hall1 = singles.tile([128, NCORES, N], f32)
        zAll = singles.tile([128, PT, N], bf16)
        lmsb = []
        for kt in range(KT):
            lmsb.append(singles.tile([128, N], bf16, tag=f"lm{kt}",
                                     name=f"lm{kt}"))

        # dram buffers for collectives
        hb0 = dpool.tile([128, N], f32)
        hg0 = dpool.tile([NCORES, 128, N], f32, addr_space="Shared")
        hb1 = dpool.tile([128, N], f32)
        hg1 = dpool.tile([NCORES, 128, N], f32, addr_space="Shared")
        zfc = dpool.tile([128, PT, N], bf16)
        zr = dpool.tile([128, PT, N], bf16, addr_space="Shared")

        def feat_branch(xt, w2t, w1p, NB, L, hsb, tagp, two_per_mm):
            """One feat_proj branch: h[k, n] = sum_l w1[k,l] sum_c w2[k,c] x[n,l,c].
            Fills hsb.  Returns per-kt last-DVE-scratch tiles (prefetch gates)."""
            # w2 as full [128, H] row tiles: 2KB DMA lines, lhsT sliced per kt
            w2sb = []
            for ci in range(NB):
                t = brp.tile([128, H], bf16, tag=f"{tagp}w2_{ci}",
                             name=f"{tagp}w2_{ci}")
                nc.sync.dma_start(out=t, in_=w2t[ci * 128:(ci + 1) * 128, :])
                w2sb.append(t)
                if ci == 0:
                    # x chunks right after the first w2 row so kt=0 can start
                    xsb = []
                    for cj in range(NB):
                        xx = brp.tile([128, NS, L], bf16, tag=f"{tagp}x{cj}",
                                      name=f"{tagp}x{cj}")
                        nc.sync.dma_start(
                            out=xx, in_=xt[cj * 128:(cj + 1) * 128, :, :])
                        xsb.append(xx)
            w1sb = []
            for kt in range(KT):
                t = brp.tile([128, L], f32, tag=f"{tagp}w1_{kt}",
                             name=f"{tagp}w1_{kt}")
                nc.sync.dma_start(out=t, in_=w1p[kt * 128:(kt + 1) * 128, :])
                touch(t)
                w1sb.append(t)
            gates = []
            for kt in range(KT):
                # all 8 samples accumulate concurrently (8 PSUM banks) so one
                # LDWEIGHTS serves 8 (or 4) matmuls
                if two_per_mm:
                    vs = []
                    for j in range(4):
                        v = pspool.tile([128, 2, L], f32, tag=f"v{j}",
                                        name=f"{tagp}v{kt}_{j}")
                        vs.append(v)
                    for ci in range(NB):
                        for j in range(4):
                            nc.tensor.matmul(
                                out=vs[j][:, :, :],
                                lhsT=w2sb[ci][:, kt * 128:(kt + 1) * 128],
                                rhs=xsb[ci][:, 2 * j:2 * j + 2, :],
                                start=(ci == 0),
                                stop=(ci == NB - 1),
                            )
                    ts = None
                    for j in range(4):
                        for s in range(2):
                            n = 2 * j + s
                            col = kt * 8 + n
                            ts = scr.tile([128, L], f32, tag="ascr",
                                          name=f"{tagp}as{kt}_{j}_{s}")
                            nc.vector.scalar_tensor_tensor(
                                out=ts[:, :], in0=vs[j][:, s, :],
                                scalar=1.0, in1=w1sb[kt][:, :],
                                op0=ALU.mult, op1=ALU.mult,
                                accum_out=hsb[:, col:col + 1])
                else:
                    vs = []
                    for j in range(8):
                        v = pspool.tile([128, L], f32, tag=f"v{j}",
                                        name=f"{tagp}v{kt}_{j}")
                        vs.append(v)
                    for ci in range(NB):
                        for j in range(8):
                            nc.tensor.matmul(
                                out=vs[j][:, :],
                                lhsT=w2sb[ci][:, kt * 128:(kt + 1) * 128],
                                rhs=xsb[ci][:, j, :],
                                start=(ci == 0),
                                stop=(ci == NB - 1),
                            )
                    ts = None
                    for j in range(8):
                        col = kt * 8 + j
                        ts = scr.tile([128, L], f32, tag="ascr",
                                      name=f"{tagp}as{kt}_{j}")
                        nc.vector.scalar_tensor_tensor(
                            out=ts[:, :], in0=vs[j][:, :],
                            scalar=1.0, in1=w1sb[kt][:, :],
                            op0=ALU.mult, op1=ALU.mult,
                            accum_out=hsb[:, col:col + 1])
                gates.append(ts)
            return gates

        def branch_stats(hall, gsb, bsb, pref):
            """Batched biased batch-norm stats over all 64 samples.
            Returns (a, nb) each [128, 8] (col = kt): a = 0.5*gamma*rstd,
            nb = 0.5*beta - a*mu."""
            s1 = stx.tile([128, KT], f32, name=f"{pref}s1")
            sq = stx.tile([128, KT], f32, name=f"{pref}sq")
            for kt in range(KT):
                hs = hall[:, :, kt * 8:kt * 8 + 8]
                t0 = scr.tile([128, N], f32, tag="ttrscr", name=f"{pref}t0_{kt}")
                t03 = t0[:].rearrange("p (g n) -> p g n", g=NCORES)
                nc.vector.scalar_tensor_tensor(
                    out=t03, in0=hs, scalar=0.0, in1=hs,
                    op0=ALU.add, op1=ALU.bypass,
                    accum_out=s1[:, kt:kt + 1])
                t1 = scr.tile([128, N], f32, tag="ttrscr", name=f"{pref}t1_{kt}")
                t13 = t1[:].rearrange("p (g n) -> p g n", g=NCORES)
                nc.vector.scalar_tensor_tensor(
                    out=t13, in0=hs, scalar=1.0, in1=hs,
                    op0=ALU.mult, op1=ALU.mult,
                    accum_out=sq[:, kt:kt + 1])
            mu = stx.tile([128, KT], f32, name=f"{pref}mu")
            nc.vector.tensor_scalar_mul(mu, s1, 1.0 / N)
            msq = stx.tile([128, KT], f32, name=f"{pref}msq")
            nc.vector.tensor_mul(msq, mu, mu)
            var = stx.tile([128, KT], f32, name=f"{pref}var")
            nc.vector.scalar_tensor_tensor(
                out=var, in0=sq, scalar=1.0 / N, in1=msq,
                op0=ALU.mult, op1=ALU.subtract)
            sd = stx.tile([128, KT], f32, name=f"{pref}sd")
            nc.scalar.activation(out=sd, in_=var, func=ACTF.Sqrt,
                                 bias=epssb, scale=1.0)
            rstd = stx.tile([128, KT], f32, name=f"{pref}rstd")
            nc.vector.reciprocal(out=rstd, in_=sd)
            a = stx.tile([128, KT], f32, name=f"{pref}a")
            nc.vector.tensor_mul(a, rstd, gsb)
            amu = stx.tile([128, KT], f32, name=f"{pref}amu")
            nc.vector.tensor_mul(amu, a, mu)
            nb = stx.tile([128, KT], f32, name=f"{pref}nb")
            nc.vector.tensor_sub(nb, bsb, amu)
            return a, nb

        ps_ctx = tc.tile_pool(name="ps", bufs=1, space="PSUM")
        pspool = ps_ctx.__enter__()

        with tc.tile_pool(name="br1", bufs=1) as br1:
            with tc.tile_pool(name="br0", bufs=1) as br0:
                # ---- stage A: branch 0 (bigger) FIRST ----
                brp = br0
                g0 = feat_branch(x0t, w2_0t, w1_0, NB0, L0, h0sb, "b0", False)

                # small resident loads on the scalar ring (sync ring stays
                # clean for the x/w2/w1 stage-A feeds)
                nc.scalar.dma_start(out=gam0sb, in_=gam0[:, :])
                nc.scalar.dma_start(out=bet0sb, in_=bet0[:, :])
                nc.scalar.dma_start(out=gam1sb, in_=gam1[:, :])
                nc.scalar.dma_start(out=bet1sb, in_=bet1[:, :])
                nc.scalar.dma_start(out=masksb, in_=mask[:, :])
                for _t in (gam0sb, bet0sb, gam1sb, bet1sb, masksb):
                    touch(_t)
                nc.vector.memset(epssb, BN_EPS)

                # embed-weight prefetch: one tile per b0 kt boundary
                for kt in range(KT):
                    ewsb[kt] = gated_dma(
                        ewp, [128, W], f8e3,
                        ewt[kt * 128:(kt + 1) * 128, :],
                        f"ew{kt}", gate=g0[kt])

                # gather h0 while branch 1 computes (SWDGE path: off the
                # bulk HWDGE rings)
                nc.gpsimd.dma_start(out=hb0, in_=h0sb[:, :])
                nc.gpsimd.collective_compute(
                    "AllGather", ALU.bypass,
                    replica_groups=[list(range(NCORES))],
                    ins=[hb0[:].opt()], outs=[hg0[:].opt()])
                nc.gpsimd.dma_start(
                    out=hall0[:, :, :], in_=hg0[:].rearrange("g p n -> p g n"))
                touch(hall0[:, 0, :])

                brp = br1
                g1 = feat_branch(x1t, w2_1t, w1_1, NB1, L1, h1sb, "b1", True)

                # h1 gather is the critical-path collective: sync-ring store
                # (drained by now) for the lowest latency
                nc.sync.dma_start(out=hb1, in_=h1sb[:, :])
                nc.gpsimd.collective_compute(
                    "AllGather", ALU.bypass,
                    replica_groups=[list(range(NCORES))],
                    ins=[hb1[:].opt()], outs=[hg1[:].opt()])

            # ---- br0 freed: cwt row tiles land in its space ----
            ps_ctx.__exit__(None, None, None)
            with tc.tile_pool(name="cpool", bufs=1) as cpool:
                cwsb = []
                for kt in range(KT):
                    # second half of the embed prefetch + cwt, paced by
                    # branch-1 compute progress
                    ewsb[8 + kt] = gated_dma(
                        ewp, [128, W], f8e3,
                        ewt[(8 + kt) * 128:(9 + kt) * 128, :],
                        f"ew{8 + kt}", gate=g1[kt])
                    cwsb.append(gated_dma(
                        cpool, [128, P], bf16,
                        cwt[kt * 128:(kt + 1) * 128, :],
                        f"cw{kt}"))
                fcbsb = cpool.tile([1, P], bf16)
                nc.scalar.dma_start(out=fcbsb, in_=fcb[:, :])
                maskrsb = cpool.tile([1, N], bf16)
                nc.scalar.dma_start(out=maskrsb, in_=maskrow[:, :])
                eb2sb = cpool.tile([1, W], bf16)
                nc.scalar.dma_start(out=eb2sb, in_=eb2[:, :])
                ones1 = cpool.tile([1, N], bf16)
                nc.vector.memset(ones1, 1.0)

                # branch-0 stats hide under branch 1's compute / h1 gather
                a0, nb0 = branch_stats(hall0, gam0sb, bet0sb, "s0_")

                nc.sync.dma_start(
                    out=hall1[:, :, :], in_=hg1[:].rearrange("g p n -> p g n"))
                touch(hall1[:, 0, :])

                # PE warm-up chain gated on hall1's arrival: spans the BN DVE
                # window so stage C's matmuls start at full clock
                warmsrc = cpool.tile([128, 256], bf16, name="warmsrc")
                nc.vector.memset(warmsrc, 0.0)
                nc.vector.tensor_copy(out=warmsrc[:, 0:1], in_=hall1[:, 0, 0:1])

                a1, nb1 = branch_stats(hall1, gam1sb, bet1sb, "s1_")
                cc = stx.tile([128, KT], f32, name="ccomb")
                nc.vector.tensor_add(cc, nb0, nb1)

                # ---- BN latent ops, then stage C in two kt-major passes ----
                with tc.tile_pool(name="zps", bufs=1, space="PSUM") as zps, \
                     tc.tile_pool(name="zsp", bufs=2) as zsp:
                    # shares tag zc0 so the 8 zc chains + warm tile fit 8 banks
                    wt = zps.tile([128, 256], f32, tag="zc0", name="warmps")
                    for i in range(16):
                        nc.tensor.matmul(out=wt[:, :], lhsT=warmsrc[:, 0:128],
                                         rhs=warmsrc[:, :], start=True,
                                         stop=True)
                    for kt in range(KT):
                        t1 = scr.tile([128, N], f32, tag="latscr",
                                      name=f"lat1_{kt}")
                        nc.vector.tensor_scalar(
                            out=t1[:].rearrange("p (g n) -> p g n", g=NCORES),
                            in0=hall1[:, :, kt * 8:kt * 8 + 8],
                            scalar1=a1[:, kt:kt + 1], scalar2=cc[:, kt:kt + 1],
                            op0=ALU.mult, op1=ALU.add)
                        t2 = scr.tile([128, N], f32, tag="latscr2",
                                      name=f"lat2_{kt}")
                        nc.vector.scalar_tensor_tensor(
                            out=t2[:].rearrange("p (g n) -> p g n", g=NCORES),
                            in0=hall0[:, :, kt * 8:kt * 8 + 8],
                            scalar=a0[:, kt:kt + 1],
                            in1=t1[:].rearrange("p (g n) -> p g n", g=NCORES),
                            op0=ALU.mult, op1=ALU.add)
                        nc.vector.tensor_mul(lmsb[kt], t2, masksb)
                    for half in range(2):
                        zsbuf = zsp.tile([128, 8, N], bf16, tag="zst",
                                         name=f"zst{half}")
                        zcs = []
                        for j in range(8):
                            pt = half * 8 + j
                            t = zps.tile([128, N], f32, tag=f"zc{j}",
                                         name=f"zc{pt}")
                            zcs.append(t)
                        for kt in range(KT):
                            for j in range(8):
                                pt = half * 8 + j
                                nc.tensor.matmul(
                                    out=zcs[j][:, :],
                                    lhsT=cwsb[kt][:, pt * 128:(pt + 1) * 128],
                                    rhs=lmsb[kt][:, :],
                                    start=(kt == 0), stop=False,
                                )
                        for j in range(8):
                            pt = half * 8 + j
                            nc.tensor.matmul(
                                out=zcs[j][:, :],
                                lhsT=fcbsb[:, pt * 128:(pt + 1) * 128],
                                rhs=maskrsb[:, :], start=False, stop=True)
                            nc.vector.tensor_copy(out=zsbuf[:, j, :],
                                                  in_=zcs[j][:, :])
                        # partition-major store: 1KB contiguous per partition
                        nc.sync.dma_start(
                            out=zfc[:, half * 8:(half + 1) * 8, :],
                            in_=zsbuf[:, :, :])

                nc.gpsimd.collective_compute(
                    "AllReduce", ALU.add,
                    replica_groups=[list(range(NCORES))],
                    ins=[zfc[:].opt()], outs=[zr[:].opt()])
                # direct 2KB-line reload, no rearrange needed
                nc.sync.dma_start(out=zAll[:, :, :], in_=zr[:, :, :])
                touch(zAll[:, 0, :])

                # ---- stage D : out = z.T @ ewt + 2*eb ----
                # block pairs run as two concurrent accumulation chains on
                # disjoint PE column groups (tile_position), nearly doubling
                # matmul throughput for the 64-wide stationary operand
                with tc.tile_pool(name="odp", bufs=2, space="PSUM") as odp, \
                     tc.tile_pool(name="osp", bufs=4) as osp:
                    for nb0_ in range(0, NBLK, 2):
                        bA, bB = nb0_, nb0_ + 1
                        bsA, bwA = bA * 512, min(512, W - bA * 512)
                        bsB, bwB = bB * 512, min(512, W - bB * 512)
                        odA = odp.tile([N, 512], f32, tag="odA",
                                       name=f"odA{bA}")
                        odB = odp.tile([128, 512], f32, tag="odB",
                                       name=f"odB{bB}")
                        for pc in range(PT):
                            nc.tensor.matmul(
                                out=odA[:, :bwA],
                                lhsT=zAll[:, pc, :],
                                rhs=ewsb[pc][:, bsA:bsA + bwA],
                                start=(pc == 0), stop=False,
                                tile_position=(0, 0))
                            nc.tensor.matmul(
                                out=odB[64:128, :bwB],
                                lhsT=zAll[:, pc, :],
                                rhs=ewsb[pc][:, bsB:bsB + bwB],
                                start=(pc == 0), stop=False,
                                tile_position=(0, 64))
                        nc.tensor.matmul(
                            out=odA[:, :bwA],
                            lhsT=ones1[:, :],
                            rhs=eb2sb[:, bsA:bsA + bwA],
                            start=False, stop=True, tile_position=(0, 0))
                        nc.tensor.matmul(
                            out=odB[64:128, :bwB],
                            lhsT=ones1[:, :],
                            rhs=eb2sb[:, bsB:bsB + bwB],
                            start=False, stop=True, tile_position=(0, 64))
                        osbA = osp.tile([N, 512], f32, tag="osbA",
                                        name=f"osbA{bA}")
                        nc.vector.tensor_copy(out=osbA[:, :bwA],
                                              in_=odA[:, :bwA])
                        nc.sync.dma_start(out=out[:, bsA:bsA + bwA],
                                          in_=osbA[:, :bwA])
                        osbB = osp.tile([128, 512], f32, tag="osbB",
                                        name=f"osbB{bB}")
                        nc.vector.tensor_copy(out=osbB[64:128, :bwB],
                                              in_=odB[64:128, :bwB])
                        nc.sync.dma_start(out=out[:, bsB:bsB + bwB],
                                          in_=osbB[64:128, :bwB])

    nc.compile()
    return nc


def _host_prep(x0, x1, w1_0, w2_0, gamma0, beta0, w1_1, w2_1, gamma1, beta1,
               shared_w, fc_w, fc_b, embed_w, embed_b, indices):
    import ml_dtypes
    f = np.float32
    bf = ml_dtypes.bfloat16
    f8 = ml_dtypes.float8_e3m4
    x0t = np.ascontiguousarray(x0.transpose(2, 0, 1)).astype(bf)   # [1024, 64, 257]
    x1t = np.ascontiguousarray(x1.transpose(2, 0, 1)).astype(bf)   # [768, 64, 197]
    w2_0t = np.ascontiguousarray(w2_0.T).astype(bf)
    w2_1t = np.ascontiguousarray(w2_1.T).astype(bf)
    gam0 = np.ascontiguousarray((gamma0 * 0.5).reshape(8, 128).T, dtype=f)
    bet0 = np.ascontiguousarray((beta0 * 0.5).reshape(8, 128).T, dtype=f)
    gam1 = np.ascontiguousarray((gamma1 * 0.5).reshape(8, 128).T, dtype=f)
    bet1 = np.ascontiguousarray((beta1 * 0.5).reshape(8, 128).T, dtype=f)
    inv_s = 1.0 / EW_SCALE
    swt = shared_w.T.astype(f) * inv_s                            # [1024, 2048]
    fcwt = fc_w.T.astype(f) * inv_s                               # [1024, 16384]
    ewt_pad = np.zeros((P, NCORES * W), dtype=f8)
    ewt_pad[:, :KE] = np.clip(embed_w.T.astype(f) * EW_SCALE,
                              -30.0, 30.0).astype(f8)
    eb2_pad = np.zeros((1, NCORES * W), dtype=bf)
    eb2_pad[0, :KE] = (2.0 * embed_b).astype(bf)

    idx = np.asarray(indices).astype(np.int64)
    in_maps = []
    for i in range(NCORES):
        m = (idx == i).astype(f)
        in_maps.append({
            "x0t": np.ascontiguousarray(x0t[:, i * NS:(i + 1) * NS, :]),
            "x1t": np.ascontiguousarray(x1t[:, i * NS:(i + 1) * NS, :]),
            "w2_0t": w2_0t,
            "w2_1t": w2_1t,
            "w1_0": np.ascontiguousarray(w1_0, dtype=f),
            "w1_1": np.ascontiguousarray(w1_1, dtype=f),
            "gam0": gam0, "bet0": bet0, "gam1": gam1, "bet1": bet1,
            "cwt": np.ascontiguousarray(swt + fcwt[:, i * P:(i + 1) * P]).astype(bf),
            "fcb": np.ascontiguousarray(
                (fc_b[i * P:(i + 1) * P] * inv_s).reshape(1, P)).astype(bf),
            "maskrow": np.ascontiguousarray(m.reshape(1, N)).astype(bf),
            "mask": np.ascontiguousarray(np.broadcast_to(m, (128, N))),
            "ewt": np.ascontiguousarray(ewt_pad[:, i * W:(i + 1) * W]),
            "eb2": np.ascontiguousarray(eb2_pad[:, i * W:(i + 1) * W]),
        })
    return in_maps


def kernel(**inputs):
    if "/opt/trn_rl_repo" not in sys.path:
        sys.path.insert(0, "/opt/trn_rl_repo")
    from concourse.bass_utils import run_bass_kernel_spmd

    in_maps = _host_prep(**inputs)
    if "nc" not in _CACHE:
        _CACHE["nc"] = _build_nc()
    nc = _CACHE["nc"]
    res = run_bass_kernel_spmd(nc, in_maps, core_ids=list(range(NCORES)))
    outs = [np.asarray(res.results[i]["out"]) for i in range(NCORES)]
    full = np.concatenate(outs, axis=1)[:, :KE]
    return np.ascontiguousarray(full, dtype=np.float32)


if __name__ == "__main__":
    sys.path.insert(0, os.path.dirname(os.path.abspath(__file__)))
    import reference
    inputs = {k: np.asarray(v) for k, v in reference.setup_inputs().items()}
    expected = np.asarray(reference.reference(**inputs))
    actual = kernel(**inputs)
    err = np.abs(actual - expected).max() / (np.abs(expected).max() + 1e-12)
    print("Relative error:", err)
